# revision 4
# baseline (speedup 1.0000x reference)
"""Trainium2 Bass kernel for per-sample softplus + max-normalize.

reference:
    pred = softplus(x)                       # x: [128, 1, 512, 512] fp32
    m    = max(pred) per sample              # [B,1,1,1]
    out  = pred / (m if m > 1e-8 else 1.0)

Sharding: pure data parallel over the batch dim -- 16 samples per core
on 8 cores. Each sample (262144 elements) is laid out on SBUF as
[128 partitions, 2048].

Datapath is fp16 end-to-end (the 2e-2 rel-err budget dwarfs fp16's
~5e-4 quantization): the host casts x to fp16, the device reads/writes
fp16 HBM (halving DMA time, the bottleneck), and ACT runs exp/ln in
its 2-elem/cycle 16-bit mode (fp32 internal datapath, so ln(u+1) keeps
full precision on the +1). max(softplus(x)) == softplus-of-max is
computed from max(u)=max(exp(x)) -- one tiny ln per sample instead of
a second big reduce.

Engine budget per core (16 samples, steady state): DMA 16.8 MB ~38us
(bound), ACT exp+ln ~31us, DVE reduce+mul ~23us, gpsimd ~2us.

Notes:
 - ACT ops are OUT-OF-PLACE: dependent in-place ACT chains measure
   2-5x slower (read-write bubble); independent out-of-place ops hit
   the 2x fp16 mode.
 - out-DMAs ride the gpsimd SWDGE ring so their data-ready waits can't
   head-of-line block in-DMAs on the sync HWDGE FIFO.
"""

import numpy as np

import concourse.bacc as bacc
import concourse.tile as tile
from concourse import bass_isa, mybir
from concourse.bass_utils import run_bass_kernel_spmd

N_CORES = 8
B, C, H, W = 128, 1, 512, 512
PER = B // N_CORES            # 16 samples per core
P = 128                       # SBUF partition count
FREE = (C * H * W) // P       # 2048 elements per partition per sample
EPS = 1e-8
G = 4                         # samples per chunk

F32 = mybir.dt.float32
F16 = mybir.dt.float16

# DRAM tensor declarations (shared with bench.py's timing harness).
# Partition-major layout: [P, PER, FREE] so every DMA slice is a plain
# contiguous [P, G*FREE] block per partition (host does the transpose).
IN_SHAPE = [P, PER, FREE]
IN_DT = F16
OUT_SHAPE = [P, PER, FREE]
OUT_DT = F16


def _emit(tc: tile.TileContext, data, stats, y_d, x_d):
    nc = tc.nc
    for c0 in range(0, PER, G):
        xt = data.tile([P, G, FREE], F16, name="xt", bufs=3)
        ut = data.tile([P, G, FREE], F16, name="ut", bufs=3)
        st = data.tile([P, G, FREE], F16, name="st", bufs=3)

        nc.sync.dma_start(out=xt[:], in_=x_d[:, c0 : c0 + G])

        # u = exp(x); softplus(x) = ln(u + 1)
        nc.scalar.activation(
            out=ut[:], in_=xt[:], func=mybir.ActivationFunctionType.Exp
        )
        nc.scalar.activation(
            out=st[:],
            in_=ut[:],
            func=mybir.ActivationFunctionType.Ln,
            bias=1.0,
        )

        # per-sample max of u (exp is monotonic, so max(softplus) =
        # ln(max(u) + 1)): free-dim reduce, then cross-partition reduce
        colmax = stats.tile([P, G], F16, name="colmax")
        nc.vector.tensor_reduce(
            out=colmax[:], in_=ut[:], axis=mybir.AxisListType.X,
            op=mybir.AluOpType.max,
        )
        allmax = stats.tile([P, G], F16, name="allmax")
        nc.gpsimd.partition_all_reduce(
            allmax[:], colmax[:], channels=P, reduce_op=bass_isa.ReduceOp.max
        )
        sm = stats.tile([P, G], F32, name="sm")
        nc.scalar.activation(
            out=sm[:], in_=allmax[:], func=mybir.ActivationFunctionType.Ln,
            bias=1.0,
        )

        # safe = where(m > EPS, m, 1.0); inv = 1/safe
        mask = stats.tile([P, G], mybir.dt.uint8, name="mask")
        nc.vector.tensor_scalar(
            out=mask[:], in0=sm[:], scalar1=EPS, scalar2=None,
            op0=mybir.AluOpType.is_gt,
        )
        safe = stats.tile([P, G], F32, name="safe")
        nc.vector.memset(safe[:], 1.0)
        nc.vector.copy_predicated(out=safe[:], mask=mask[:], data=sm[:])
        inv = stats.tile([P, G], F32, name="inv")
        nc.vector.reciprocal(out=inv[:], in_=safe[:])

        for s in range(G):
            nc.vector.tensor_scalar_mul(
                out=st[:, s], in0=st[:, s], scalar1=inv[:, s : s + 1]
            )

        nc.gpsimd.dma_start(out=y_d[:, c0 : c0 + G], in_=st[:])


def _body(tc: tile.TileContext, y_d, x_d):
    with (
        tc.tile_pool(name="data", bufs=3) as data,
        tc.tile_pool(name="stats", bufs=6) as stats,
    ):
        _emit(tc, data, stats, y_d, x_d)


_compiled = None


def _steered_activation_tables():
    """Activation-table list with exp/ln visible only in sets that hold BOTH.

    The act-table chooser greedily takes the first set containing each
    function: exp -> 'exp_and_others', ln -> 'natural_log', which forces a
    ~2.7us LoadActFuncSet between every exp/ln pair. Hiding exp/ln from
    the single-function sets steers the chooser to
    'natural_log_exp_and_others' so the whole kernel needs one load.
    Set names/order (= set ids) unchanged.
    """
    from concourse.hw_specs import get_activation_tables

    def steer(arch):
        tables = get_activation_tables(arch)
        both = {
            mybir.ActivationFunctionType.Exp,
            mybir.ActivationFunctionType.Ln,
        }
        out = {}
        for name, funcs in tables.items():
            if not both.issubset(funcs):
                funcs = funcs - both
            out[name] = funcs
        return out

    return steer


def _build():
    global _compiled
    if _compiled is None:
        nc = bacc.Bacc("TRN2", target_bir_lowering=False, debug=False)
        x_d = nc.dram_tensor("x", IN_SHAPE, IN_DT, kind="ExternalInput").ap()
        y_d = nc.dram_tensor("y", OUT_SHAPE, OUT_DT, kind="ExternalOutput").ap()
        with tile.TileContext(nc) as tc:
            _body(tc, y_d, x_d)
        _compile(nc)
        _compiled = nc
    return _compiled


def _compile(nc):
    orig = bacc.get_activation_tables
    bacc.get_activation_tables = _steered_activation_tables()
    try:
        nc.compile()
    finally:
        bacc.get_activation_tables = orig


def kernel(x: np.ndarray) -> np.ndarray:
    nc = _build()
    shards = np.ascontiguousarray(
        np.asarray(x)
        .astype(np.float16)
        .reshape(N_CORES, PER, P, FREE)
        .transpose(0, 2, 1, 3)
    )
    in_maps = [{"x": shards[i]} for i in range(N_CORES)]
    res = run_bass_kernel_spmd(nc, in_maps, list(range(N_CORES)))
    out = np.stack([res.results[i]["y"] for i in range(N_CORES)])  # [NC,P,PER,FREE]
    return (
        out.transpose(0, 2, 1, 3).astype(np.float32).reshape(B, C, H, W)
    )


# revision 5
# speedup vs baseline: 1.2415x; 1.2415x over previous
"""Trainium2 Bass kernel for per-sample softplus + max-normalize.

reference:
    pred = softplus(x)                       # x: [128, 1, 512, 512] fp32
    m    = max(pred) per sample              # [B,1,1,1]
    out  = pred / (m if m > 1e-8 else 1.0)

Sharding: pure data parallel over the batch dim -- 16 samples per core
on 8 cores. Each sample (262144 elements) is laid out on SBUF as
[128 partitions, 2048].

Datapath is fp16 end-to-end (the 2e-2 rel-err budget dwarfs fp16's
~5e-4 quantization): the host casts x to fp16, the device reads/writes
fp16 HBM (halving DMA time, the bottleneck), and ACT runs exp/ln in
its 2-elem/cycle 16-bit mode (fp32 internal datapath, so ln(u+1) keeps
full precision on the +1). max(softplus(x)) == softplus-of-max is
computed from max(u)=max(exp(x)) -- one tiny ln per sample instead of
a second big reduce.

Engine budget per core (16 samples, steady state): DMA 16.8 MB ~38us
(bound), ACT exp+ln ~31us, DVE reduce+mul ~23us, gpsimd ~2us.

Notes:
 - ACT ops are OUT-OF-PLACE: dependent in-place ACT chains measure
   2-5x slower (read-write bubble); independent out-of-place ops hit
   the 2x fp16 mode.
 - out-DMAs ride the gpsimd SWDGE ring so their data-ready waits can't
   head-of-line block in-DMAs on the sync HWDGE FIFO.
"""

import numpy as np

import concourse.bacc as bacc
import concourse.tile as tile
from concourse import bass_isa, mybir
from concourse.bass_utils import run_bass_kernel_spmd

N_CORES = 8
B, C, H, W = 128, 1, 512, 512
PER = B // N_CORES            # 16 samples per core
P = 128                       # SBUF partition count
FREE = (C * H * W) // P       # 2048 elements per partition per sample
EPS = 1e-8
G = 4                         # samples per chunk

F32 = mybir.dt.float32
F16 = mybir.dt.float16

# DRAM tensor declarations (shared with bench.py's timing harness).
# Partition-major layout: [P, PER, FREE] so every DMA slice is a plain
# contiguous [P, G*FREE] block per partition (host does the transpose).
IN_SHAPE = [P, PER, FREE]
IN_DT = F16
OUT_SHAPE = [P, PER, FREE]
OUT_DT = F16


def _emit(tc: tile.TileContext, data, stats, y_d, x_d):
    nc = tc.nc
    for c0 in range(0, PER, G):
        xt = data.tile([P, G, FREE], F16, name="xt", bufs=3)
        ut = data.tile([P, G, FREE], F16, name="ut", bufs=3)
        st = data.tile([P, G, FREE], F16, name="st", bufs=3)

        nc.sync.dma_start(out=xt[:], in_=x_d[:, c0 : c0 + G])

        # u = exp(x); softplus(x) = ln(u + 1)
        nc.scalar.activation(
            out=ut[:], in_=xt[:], func=mybir.ActivationFunctionType.Exp
        )
        nc.scalar.activation(
            out=st[:],
            in_=ut[:],
            func=mybir.ActivationFunctionType.Ln,
            bias=1.0,
        )

        # per-sample max of u (exp is monotonic, so max(softplus) =
        # ln(max(u) + 1)): free-dim reduce, then cross-partition reduce
        colmax = stats.tile([P, G], F16, name="colmax")
        nc.vector.tensor_reduce(
            out=colmax[:], in_=ut[:], axis=mybir.AxisListType.X,
            op=mybir.AluOpType.max,
        )
        allmax = stats.tile([P, G], F16, name="allmax")
        nc.gpsimd.partition_all_reduce(
            allmax[:], colmax[:], channels=P, reduce_op=bass_isa.ReduceOp.max
        )
        sm = stats.tile([P, G], F32, name="sm")
        nc.scalar.activation(
            out=sm[:], in_=allmax[:], func=mybir.ActivationFunctionType.Ln,
            bias=1.0,
        )

        # safe = where(m > EPS, m, 1.0); inv = 1/safe
        mask = stats.tile([P, G], mybir.dt.uint8, name="mask")
        nc.vector.tensor_scalar(
            out=mask[:], in0=sm[:], scalar1=EPS, scalar2=None,
            op0=mybir.AluOpType.is_gt,
        )
        safe = stats.tile([P, G], F32, name="safe")
        nc.vector.memset(safe[:], 1.0)
        nc.vector.copy_predicated(out=safe[:], mask=mask[:], data=sm[:])
        inv = stats.tile([P, G], F32, name="inv")
        nc.vector.reciprocal(out=inv[:], in_=safe[:])

        for s in range(G):
            nc.vector.tensor_scalar_mul(
                out=st[:, s], in0=st[:, s], scalar1=inv[:, s : s + 1]
            )

        nc.sync.dma_start(out=y_d[:, c0 : c0 + G], in_=st[:])


def _body(tc: tile.TileContext, y_d, x_d):
    with (
        tc.tile_pool(name="data", bufs=3) as data,
        tc.tile_pool(name="stats", bufs=6) as stats,
    ):
        _emit(tc, data, stats, y_d, x_d)


_compiled = None


def _steered_activation_tables():
    """Activation-table list with exp/ln visible only in sets that hold BOTH.

    The act-table chooser greedily takes the first set containing each
    function: exp -> 'exp_and_others', ln -> 'natural_log', which forces a
    ~2.7us LoadActFuncSet between every exp/ln pair. Hiding exp/ln from
    the single-function sets steers the chooser to
    'natural_log_exp_and_others' so the whole kernel needs one load.
    Set names/order (= set ids) unchanged.
    """
    from concourse.hw_specs import get_activation_tables

    def steer(arch):
        tables = get_activation_tables(arch)
        both = {
            mybir.ActivationFunctionType.Exp,
            mybir.ActivationFunctionType.Ln,
        }
        out = {}
        for name, funcs in tables.items():
            if not both.issubset(funcs):
                funcs = funcs - both
            out[name] = funcs
        return out

    return steer


def _build():
    global _compiled
    if _compiled is None:
        nc = bacc.Bacc("TRN2", target_bir_lowering=False, debug=False)
        x_d = nc.dram_tensor("x", IN_SHAPE, IN_DT, kind="ExternalInput").ap()
        y_d = nc.dram_tensor("y", OUT_SHAPE, OUT_DT, kind="ExternalOutput").ap()
        with tile.TileContext(nc) as tc:
            _body(tc, y_d, x_d)
        _compile(nc)
        _compiled = nc
    return _compiled


def _compile(nc):
    orig = bacc.get_activation_tables
    bacc.get_activation_tables = _steered_activation_tables()
    try:
        nc.compile()
    finally:
        bacc.get_activation_tables = orig


def kernel(x: np.ndarray) -> np.ndarray:
    nc = _build()
    shards = np.ascontiguousarray(
        np.asarray(x)
        .astype(np.float16)
        .reshape(N_CORES, PER, P, FREE)
        .transpose(0, 2, 1, 3)
    )
    in_maps = [{"x": shards[i]} for i in range(N_CORES)]
    res = run_bass_kernel_spmd(nc, in_maps, list(range(N_CORES)))
    out = np.stack([res.results[i]["y"] for i in range(N_CORES)])  # [NC,P,PER,FREE]
    return (
        out.transpose(0, 2, 1, 3).astype(np.float32).reshape(B, C, H, W)
    )
